# revision 34
# baseline (speedup 1.0000x reference)
"""Equivariant rotation conv for Trainium2, 8-core batch-parallel,
vertical-Winograd F(2,3) formulation.

Computes: rotate a (128*8, 128, 3, 3) filter bank by 8 data-dependent angles
(bilinear resampling), run a 3x3 same-padded conv of x (16,128,128,128) with
all 8*128 rotated filters, then max over the 8 rotations -> (16,128,128,128).

Sharding: data-parallel over batch, 2 images per core; the filter bank is
replicated.  The rotation (a 9x9 bilinear mix, a pure function of the 8
rot_alpha scalars) and a vertical Winograd F(2,3) G-transform are folded into
the weights on the host, producing 4 transformed vertical taps x 3 horizontal
taps per rotation in bf16.  On device, per core:
  - x arrives pre-cast to bf16; per 32-row block the DVE builds 4 transformed
    row-planes (t0 = d0-d2, t1 = d1+d2, t2 = d2-d1, t3 = d1-d3 over row pairs)
    with strided-row tensor_tensor ops in the 2x bf16 mode,
  - the conv needs only 12 PE matmuls per 8 output rows (4 m-planes x 3
    horizontal taps, f32 PSUM accumulation) instead of 18 direct ones: the
    two output rows of each pair are recombined as y0 = m0+m1+m2,
    y1 = m1-m2-m3 outside the PE,
  - ACT copies each PSUM half-group (2 banks) to bf16 SBUF as soon as its 6
    matmuls land -- PSUM dep tracking is tile-granular, so the halves live
    in separate tiles (Pa/Pb) to keep the m0/m1 copy off the m2/m3 matmuls'
    critical path,
  - the DVE runs the inverse transform + running rotation max in the 2x bf16
    mode (u = m0+m1, v = m1-m2, y pair into an interleaved even/odd tile,
    one fused 8-row max),
  - the interleaved bf16 accumulator (rows already in output order) is
    DMA'd out directly as bf16 and widened to f32 on the host (lossless);
    the final block runs pair-group-major so its flush DMAs hide under the
    remaining matmuls.

Measured on trn2 (8 cores): ~697 us vs ~1030 us for the direct 9-tap bf16
kernel (PE ~97% busy, zero >300ns pipeline gaps, at the 512-col matmul
streaming roofline; DVE ~79%, ACT ~74%).  Numerics: rel_l2 ~3.7e-3 vs the
f32 reference (bf16 products, f32 PSUM accumulation, bf16 max tree; gate
is 2e-2).
"""

import numpy as np
import ml_dtypes


def _install_axon_hooks_shim():
    """Provide antenv.axon_hooks (NTFF profile hook) when the image's antenv
    lacks it, so run_bass_kernel_spmd(trace=True) works instead of crashing
    on import."""
    import contextlib
    import ctypes
    import os
    import sys
    import types

    try:
        import antenv.axon_hooks  # noqa: F401

        return
    except ImportError:
        pass

    state = {"hook": None, "resolved": False}

    def _make_hook():
        so_path = os.environ.get("AXON_PJRT_SO", "/opt/axon/libaxon_pjrt.so")
        if not os.path.exists(so_path):
            return None
        lib = ctypes.CDLL(so_path)
        if not hasattr(lib, "axon_start_nrt_profile"):
            return None
        lib.axon_start_nrt_profile.argtypes = [
            ctypes.POINTER(ctypes.c_int64),
            ctypes.c_size_t,
        ]
        lib.axon_start_nrt_profile.restype = ctypes.c_int64
        lib.axon_stop_nrt_profile.argtypes = [ctypes.c_char_p]
        lib.axon_stop_nrt_profile.restype = ctypes.c_int64

        @contextlib.contextmanager
        def _hook(output_dir, device_ids):
            import jax

            jax.devices()
            if device_ids:
                ids = (ctypes.c_int64 * len(device_ids))(*device_ids)
                rc = lib.axon_start_nrt_profile(ids, len(device_ids))
            else:
                rc = lib.axon_start_nrt_profile(None, 0)
            if rc != 0:
                raise RuntimeError(f"axon_start_nrt_profile rc={rc}")
            try:
                yield
            finally:
                n = lib.axon_stop_nrt_profile(str(output_dir).encode())
                if n < 0:
                    raise RuntimeError(f"axon_stop_nrt_profile rc={n}")
                print(f"profile: {n} file(s) written to {output_dir}")

        return _hook

    mod = types.ModuleType("antenv.axon_hooks")

    def set_axon_ntff_profile_hook(h):
        state["hook"] = h
        state["resolved"] = True

    def get_axon_ntff_profile_hook():
        if not state["resolved"]:
            state["hook"] = _make_hook()
            state["resolved"] = True
        return state["hook"]

    mod.set_axon_ntff_profile_hook = set_axon_ntff_profile_hook
    mod.get_axon_ntff_profile_hook = get_axon_ntff_profile_hook
    sys.modules["antenv.axon_hooks"] = mod


_install_axon_hooks_shim()

import concourse.bass as bass  # noqa: E402,F401
import concourse.mybir as mybir  # noqa: E402
from concourse import bacc  # noqa: E402
from concourse.bass_utils import run_bass_kernel_spmd  # noqa: E402
from concourse.tile import TileContext  # noqa: E402

F32 = mybir.dt.float32
BF16 = mybir.dt.bfloat16
BF16NP = ml_dtypes.bfloat16

B, CIN, H, W = 16, 128, 128, 128
R, O, K = 8, 128, 3
NCORES = 8
BL = B // NCORES   # images per core
RB = 32            # output rows per block
NPAIR = RB // 2    # winograd row pairs per block
NG = NPAIR // 4    # matmul groups (4 pairs = 8 output rows) per block
NBLK = H // RB

ADD = mybir.AluOpType.add
SUB = mybir.AluOpType.subtract
MAX = mybir.AluOpType.max

_TRACE = False
LAST_RESULTS = None
_NC_CACHE = {}


def _rot_mats(rot_alpha):
    """Per-rotation 9x9 bilinear resampling matrices, matching the reference
    F.grid_sample(align_corners=True, zeros) tap logic exactly.

    M[r, p, q]: coefficient of original tap q = (qy*3+qx) in rotated tap
    p = (py*3+px)."""
    M = np.zeros((R, 9, 9), np.float64)
    lin = np.linspace(-1.0, 1.0, K)
    for r in range(R):
        ang = float(rot_alpha[r]) * (np.pi / 4.0) * r
        c, s = np.cos(ang), np.sin(ang)
        for a in range(K):          # output row (gy = lin[a])
            for b in range(K):      # output col (gx = lin[b])
                gx, gy = lin[b], lin[a]
                xs = c * gx - s * gy
                ys = s * gx + c * gy
                ix = (xs + 1.0) * 0.5 * (K - 1)
                iy = (ys + 1.0) * 0.5 * (K - 1)
                x0 = int(np.floor(ix))
                y0 = int(np.floor(iy))
                wx, wy = ix - x0, iy - y0
                p = a * K + b
                for yi, xi, wt in (
                    (y0, x0, (1 - wy) * (1 - wx)),
                    (y0, x0 + 1, (1 - wy) * wx),
                    (y0 + 1, x0, wy * (1 - wx)),
                    (y0 + 1, x0 + 1, wy * wx),
                ):
                    if 0 <= yi < K and 0 <= xi < K:
                        M[r, p, yi * K + xi] += wt
    return M.astype(np.float32)


def _build():
    nc = bacc.Bacc(trn_type="TRN2")
    xs = nc.dram_tensor("xs", [BL, CIN, H, W], BF16, kind="ExternalInput")
    # wt[r, i, (j*3+kx)*O + o]: vertical-Winograd-transformed rotated filters
    wt = nc.dram_tensor("wt", [R, CIN, 12 * O], BF16, kind="ExternalInput")
    # output stays bf16 on device (the max accumulator is bf16); the host
    # widens to f32 losslessly after the gather
    y = nc.dram_tensor("y", [BL, O, H, W], BF16, kind="ExternalOutput")

    with TileContext(nc) as tc:
        with (
            tc.tile_pool(name="wpool", bufs=1) as wpool,
            tc.tile_pool(name="xpool", bufs=1) as xpool,
            tc.tile_pool(name="cpool", bufs=1) as cpool,
            tc.tile_pool(name="psum", bufs=1, space="PSUM") as ppool,
        ):
            # transformed weights: [cin, r, 12, O], all rotations resident
            wtile = wpool.tile([128, R, 12, O], BF16, name="wtile", tag="wt")

            # PE warm-up: dependency-free matmuls on a scratch tile keep the
            # PE busy from ~0.5us until the first real matmul so the HAM
            # clock gate reaches 8/8 before real work.
            dum_lhs = wpool.tile([128, 128], BF16, name="dum_lhs", tag="dum")
            nc.vector.memset(dum_lhs[:, :], 0.0)
            dum_ps = ppool.tile([128, 128], F32, name="dum_ps", tag="Pa0")
            for _ in range(150):
                nc.tensor.matmul(
                    dum_ps[:, :], dum_lhs[:, :], dum_lhs[:, :],
                    start=True, stop=True,
                )

            def load_weights(rr):
                # the sync DMA queue is serial: rotation 0 goes first, then
                # the first x block, then the remaining rotations
                for r in rr:
                    nc.sync.dma_start(out=wtile[:, r, :, :], in_=wt[r, :, :])

            load_weights([0])

            # x staging ping-pong: [34 rows, 130 cols] bf16, halo zeroed once
            xmm2 = [
                xpool.tile([128, RB + 2, W + 2], BF16, name=f"xmm{i}", tag=f"xmm{i}")
                for i in range(2)
            ]
            for i in range(2):
                nc.gpsimd.memset(xmm2[i][:, :, :], 0.0)

            # winograd row planes: [16 pairs, 130] x 4, double buffered
            tst = [
                [
                    xpool.tile([128, NPAIR, W + 2], BF16, name=f"t{p}{j}", tag=f"t{p}{j}")
                    for j in range(4)
                ]
                for p in range(2)
            ]

            def load_x(g, b, blk, chunks=None):
                h0 = blk * RB
                r0 = max(h0 - 1, 0)
                r1 = min(h0 + RB + 1, H)
                xmm = xmm2[g % 2]
                if g >= 2:
                    # restore halo-row zeros clobbered by the previous user
                    if blk == 0:
                        nc.gpsimd.memset(xmm[:, 0:1, :], 0.0)
                    elif blk == NBLK - 1:
                        nc.gpsimd.memset(xmm[:, RB + 1 : RB + 2, :], 0.0)
                d0 = r0 - (h0 - 1)
                cuts = [0, r1 - r0] if chunks is None else chunks
                for k in range(len(cuts) - 1):
                    a, c = cuts[k], cuts[k + 1]
                    # x loads ride the otherwise-idle gpsimd queue so they
                    # overlap the weight DMAs (head) and y flushes (sync)
                    nc.gpsimd.dma_start(
                        out=xmm[:, d0 + a : d0 + c, 1 : W + 1],
                        in_=xs[b, :, r0 + a : r0 + c, :],
                    )
                return xmm

            def transform(g, xmm, pair0=0, pair1=NPAIR):
                # pair s covers output rows 2s, 2s+1 of the block;
                # d_k = xmm row 2s+k (xmm row i = image row h0-1+i)
                t = tst[g % 2]
                d = [
                    xmm[:, 2 * pair0 + k : min(2 * pair1 + k, RB + 2) : 2, :]
                    for k in range(4)
                ]
                sl = slice(pair0, pair1)
                nc.vector.tensor_tensor(t[0][:, sl, :], d[0], d[2], SUB)
                nc.vector.tensor_tensor(t[1][:, sl, :], d[1], d[2], ADD)
                nc.vector.tensor_tensor(t[2][:, sl, :], d[2], d[1], SUB)
                nc.vector.tensor_tensor(t[3][:, sl, :], d[1], d[3], SUB)

            # psum: 2 phases x 2 half-tiles [2 m-planes, 4 pairs, W] f32;
            # separate tiles (tags) so the m0/m1 copy never aliases the
            # m2/m3 matmuls — PSUM dep tracking is tile-granular
            Pa = [
                ppool.tile([128, 2, 4, W], F32, name=f"Pa{p}", tag=f"Pa{p}")
                for p in range(2)
            ]
            Pb = [
                ppool.tile([128, 2, 4, W], F32, name=f"Pb{p}", tag=f"Pb{p}")
                for p in range(2)
            ]
            mba = [
                cpool.tile([128, 2, 4, W], BF16, name=f"mba{p}", tag=f"mba{p}")
                for p in range(2)
            ]
            mbb = [
                cpool.tile([128, 2, 4, W], BF16, name=f"mbb{p}", tag=f"mbb{p}")
                for p in range(2)
            ]
            uv = [
                [
                    cpool.tile([128, 4, W], BF16, name=f"uv{p}{i}", tag=f"uv{p}{i}")
                    for i in range(2)
                ]
                for p in range(2)
            ]
            # y-pair staging interleaved even/odd, so one fused max per group
            yI = [
                cpool.tile([128, 4, 2, W], BF16, name=f"yI{p}", tag=f"yI{p}")
                for p in range(2)
            ]
            # block accumulator, rows already in output order
            accI = [
                cpool.tile([128, NPAIR, 2, W], BF16, name=f"accI{p}", tag=f"accI{p}")
                for p in range(2)
            ]


            gctr = [0]

            def conv_group(g, r, sp):
                ph = gctr[0] % 2
                gctr[0] += 1
                t = tst[g % 2]

                def mms(pt, js):
                    for jj, j in enumerate(js):
                        for kx in range(3):
                            nc.tensor.matmul(
                                pt[:, jj, :, :],
                                wtile[:, r, j * 3 + kx, :],
                                t[j][:, 4 * sp : 4 * sp + 4, kx : kx + W],
                                start=(kx == 0), stop=(kx == 2),
                            )

                # copy each PSUM half as soon as its matmuls finish: the
                # m0/m1 copy overlaps the j=2,3 matmuls, and the op gating
                # the next phase's matmuls shrinks to a half-copy
                mms(Pa[ph], (0, 1))
                nc.scalar.copy(mba[ph][:, :, :, :], Pa[ph][:, :, :, :])
                mms(Pb[ph], (2, 3))
                nc.scalar.copy(mbb[ph][:, :, :, :], Pb[ph][:, :, :, :])
                m0, m1 = mba[ph][:, 0], mba[ph][:, 1]
                m2, m3 = mbb[ph][:, 0], mbb[ph][:, 1]
                u, v = uv[ph]
                acc = accI[g % 2][:, 4 * sp : 4 * sp + 4, :, :]
                yt = acc if r == 0 else yI[ph]
                nc.vector.tensor_tensor(u[:, :, :], m0, m1, ADD)
                nc.vector.tensor_tensor(v[:, :, :], m1, m2, SUB)
                nc.vector.tensor_tensor(yt[:, :, 0, :], u[:, :, :], m2, ADD)
                nc.vector.tensor_tensor(yt[:, :, 1, :], v[:, :, :], m3, SUB)
                if r > 0:
                    nc.vector.tensor_tensor(acc, acc, yt[:, :, :, :], MAX)

            def flush_block(g, b, blk, p0=0, p1=NPAIR, eng=None):
                p = g % 2
                h0 = blk * RB + 2 * p0
                (eng or nc.sync).dma_start(
                    out=y[b, :, h0 : h0 + 2 * (p1 - p0), :],
                    in_=accI[p][:, p0:p1, :, :].rearrange("i s e w -> i (s e) w"),
                )

            blocks = [(g, divmod(g, NBLK)) for g in range(BL * NBLK)]
            # first block: land the first 11 rows early so transform+matmuls
            # for the leading pairs start before the whole block arrives
            xmm0 = load_x(0, *blocks[0][1], chunks=[0, 10, 33])
            load_weights(range(1, R))
            transform(0, xmm0, 0, 4)
            transform(0, xmm0, 4, NPAIR)
            for g, (b, blk) in blocks[:-1]:
                for r in range(R):
                    if r == 1 and g + 1 < len(blocks):
                        nb, nblk = blocks[g + 1][1]
                        transform(g + 1, load_x(g + 1, nb, nblk))
                    if r == 2 and g > 0:
                        flush_block(g - 1, *blocks[g - 1][1])
                    for sp in range(NG):
                        conv_group(g, r, sp)
            # last block runs pair-group-major: each 8-row strip finishes all
            # rotations ~21us before the end, so its flush + output DMA hide
            # under the remaining strips' matmuls instead of trailing them
            g, (b, blk) = blocks[-1]
            for sp in range(NG):
                for r in range(R):
                    conv_group(g, r, sp)
                    if sp == 0 and r == 2:
                        flush_block(g - 1, *blocks[g - 1][1])
                # alternate queues so the two sliver transfers overlap
                flush_block(g, b, blk, 4 * sp, 4 * sp + 2,
                            eng=nc.sync if sp % 2 == 0 else nc.gpsimd)
                flush_block(g, b, blk, 4 * sp + 2, 4 * sp + 4,
                            eng=nc.gpsimd if sp % 2 == 0 else nc.sync)
    nc.finalize()
    return nc


def _get_nc():
    if "wino" not in _NC_CACHE:
        _NC_CACHE["wino"] = _build()
    return _NC_CACHE["wino"]


def _prep_weights(weight, rot_alpha):
    """Rotate the filter bank by the 8 angles and fold the vertical Winograd
    F(2,3) G-transform in; returns [R, CIN, 12*O] bf16."""
    M = _rot_mats(rot_alpha)
    w_r = (
        weight.reshape(O, R, CIN, 9).transpose(1, 0, 2, 3).astype(np.float64)
    )  # (R, O, I, 9)
    rot = np.einsum("rpq,roiq->roip", M.astype(np.float64), w_r)
    rot = rot.reshape(R, O, CIN, 3, 3)  # (ky, kx)
    G = np.array(
        [[1, 0, 0], [0.5, 0.5, 0.5], [0.5, -0.5, 0.5], [0, 0, 1]], np.float64
    )
    gp = np.einsum("jk,roikx->rijxo", G, rot)  # (R, I, 4, 3, O)
    return np.ascontiguousarray(
        gp.reshape(R, CIN, 12 * O).astype(np.float32).astype(BF16NP)
    )


def kernel(x, weight, rot_alpha):
    global LAST_RESULTS
    x = np.asarray(x, np.float32)
    weight = np.asarray(weight, np.float32)
    rot_alpha = np.asarray(rot_alpha, np.float32)

    wt = _prep_weights(weight, rot_alpha)
    xb = np.ascontiguousarray(x.astype(BF16NP))

    nc = _get_nc()
    in_maps = [
        {"xs": np.ascontiguousarray(xb[c * BL : (c + 1) * BL]), "wt": wt}
        for c in range(NCORES)
    ]
    try:
        res = run_bass_kernel_spmd(nc, in_maps, list(range(NCORES)), trace=_TRACE)
    except Exception:
        # One retry (without tracing): a failed compile or an aborted run can
        # leave a NeuronCore transiently wedged; the next attempt recovers.
        res = run_bass_kernel_spmd(nc, in_maps, list(range(NCORES)), trace=False)
    LAST_RESULTS = res
    return np.concatenate(
        [res.results[c]["y"] for c in range(NCORES)], axis=0
    ).astype(np.float32)


# revision 36
# speedup vs baseline: 1.0005x; 1.0005x over previous
"""Equivariant rotation conv for Trainium2, 8-core batch-parallel,
vertical-Winograd F(2,3) formulation.

Computes: rotate a (128*8, 128, 3, 3) filter bank by 8 data-dependent angles
(bilinear resampling), run a 3x3 same-padded conv of x (16,128,128,128) with
all 8*128 rotated filters, then max over the 8 rotations -> (16,128,128,128).

Sharding: data-parallel over batch, 2 images per core; the filter bank is
replicated.  The rotation (a 9x9 bilinear mix, a pure function of the 8
rot_alpha scalars) and a vertical Winograd F(2,3) G-transform are folded into
the weights on the host, producing 4 transformed vertical taps x 3 horizontal
taps per rotation in bf16.  On device, per core:
  - x arrives pre-cast to bf16; per 32-row block the DVE builds 4 transformed
    row-planes (t0 = d0-d2, t1 = d1+d2, t2 = d2-d1, t3 = d1-d3 over row pairs)
    with strided-row tensor_tensor ops in the 2x bf16 mode,
  - the conv needs only 12 PE matmuls per 8 output rows (4 m-planes x 3
    horizontal taps, f32 PSUM accumulation) instead of 18 direct ones: the
    two output rows of each pair are recombined as y0 = m0+m1+m2,
    y1 = m1-m2-m3 outside the PE,
  - ACT copies each PSUM half-group (2 banks) to bf16 SBUF as soon as its 6
    matmuls land -- PSUM dep tracking is tile-granular, so the halves live
    in separate tiles (Pa/Pb) to keep the m0/m1 copy off the m2/m3 matmuls'
    critical path,
  - the DVE runs the inverse transform + running rotation max in the 2x bf16
    mode (u = m0+m1, v = m1-m2, y pair into an interleaved even/odd tile,
    one fused 8-row max),
  - the interleaved bf16 accumulator (rows already in output order) is
    DMA'd out directly as bf16 and widened to f32 on the host (lossless);
    the final block runs pair-group-major so its flush DMAs hide under the
    remaining matmuls.

Measured on trn2 (8 cores): ~697 us vs ~1030 us for the direct 9-tap bf16
kernel (PE ~97% busy, zero >300ns pipeline gaps, at the 512-col matmul
streaming roofline; DVE ~79%, ACT ~74%).  Numerics: rel_l2 ~3.7e-3 vs the
f32 reference (bf16 products, f32 PSUM accumulation, bf16 max tree; gate
is 2e-2).
"""

import numpy as np
import ml_dtypes


def _install_axon_hooks_shim():
    """Provide antenv.axon_hooks (NTFF profile hook) when the image's antenv
    lacks it, so run_bass_kernel_spmd(trace=True) works instead of crashing
    on import."""
    import contextlib
    import ctypes
    import os
    import sys
    import types

    try:
        import antenv.axon_hooks  # noqa: F401

        return
    except ImportError:
        pass

    state = {"hook": None, "resolved": False}

    def _make_hook():
        so_path = os.environ.get("AXON_PJRT_SO", "/opt/axon/libaxon_pjrt.so")
        if not os.path.exists(so_path):
            return None
        lib = ctypes.CDLL(so_path)
        if not hasattr(lib, "axon_start_nrt_profile"):
            return None
        lib.axon_start_nrt_profile.argtypes = [
            ctypes.POINTER(ctypes.c_int64),
            ctypes.c_size_t,
        ]
        lib.axon_start_nrt_profile.restype = ctypes.c_int64
        lib.axon_stop_nrt_profile.argtypes = [ctypes.c_char_p]
        lib.axon_stop_nrt_profile.restype = ctypes.c_int64

        @contextlib.contextmanager
        def _hook(output_dir, device_ids):
            import jax

            jax.devices()
            if device_ids:
                ids = (ctypes.c_int64 * len(device_ids))(*device_ids)
                rc = lib.axon_start_nrt_profile(ids, len(device_ids))
            else:
                rc = lib.axon_start_nrt_profile(None, 0)
            if rc != 0:
                raise RuntimeError(f"axon_start_nrt_profile rc={rc}")
            try:
                yield
            finally:
                n = lib.axon_stop_nrt_profile(str(output_dir).encode())
                if n < 0:
                    raise RuntimeError(f"axon_stop_nrt_profile rc={n}")
                print(f"profile: {n} file(s) written to {output_dir}")

        return _hook

    mod = types.ModuleType("antenv.axon_hooks")

    def set_axon_ntff_profile_hook(h):
        state["hook"] = h
        state["resolved"] = True

    def get_axon_ntff_profile_hook():
        if not state["resolved"]:
            state["hook"] = _make_hook()
            state["resolved"] = True
        return state["hook"]

    mod.set_axon_ntff_profile_hook = set_axon_ntff_profile_hook
    mod.get_axon_ntff_profile_hook = get_axon_ntff_profile_hook
    sys.modules["antenv.axon_hooks"] = mod


_install_axon_hooks_shim()

import concourse.bass as bass  # noqa: E402,F401
import concourse.mybir as mybir  # noqa: E402
from concourse import bacc  # noqa: E402
from concourse.bass_utils import run_bass_kernel_spmd  # noqa: E402
from concourse.tile import TileContext  # noqa: E402

F32 = mybir.dt.float32
BF16 = mybir.dt.bfloat16
BF16NP = ml_dtypes.bfloat16

B, CIN, H, W = 16, 128, 128, 128
R, O, K = 8, 128, 3
NCORES = 8
BL = B // NCORES   # images per core
RB = 32            # output rows per block
NPAIR = RB // 2    # winograd row pairs per block
NG = NPAIR // 4    # matmul groups (4 pairs = 8 output rows) per block
NBLK = H // RB

ADD = mybir.AluOpType.add
SUB = mybir.AluOpType.subtract
MAX = mybir.AluOpType.max

_TRACE = False
LAST_RESULTS = None
_NC_CACHE = {}


def _rot_mats(rot_alpha):
    """Per-rotation 9x9 bilinear resampling matrices, matching the reference
    F.grid_sample(align_corners=True, zeros) tap logic exactly.

    M[r, p, q]: coefficient of original tap q = (qy*3+qx) in rotated tap
    p = (py*3+px)."""
    M = np.zeros((R, 9, 9), np.float64)
    lin = np.linspace(-1.0, 1.0, K)
    for r in range(R):
        ang = float(rot_alpha[r]) * (np.pi / 4.0) * r
        c, s = np.cos(ang), np.sin(ang)
        for a in range(K):          # output row (gy = lin[a])
            for b in range(K):      # output col (gx = lin[b])
                gx, gy = lin[b], lin[a]
                xs = c * gx - s * gy
                ys = s * gx + c * gy
                ix = (xs + 1.0) * 0.5 * (K - 1)
                iy = (ys + 1.0) * 0.5 * (K - 1)
                x0 = int(np.floor(ix))
                y0 = int(np.floor(iy))
                wx, wy = ix - x0, iy - y0
                p = a * K + b
                for yi, xi, wt in (
                    (y0, x0, (1 - wy) * (1 - wx)),
                    (y0, x0 + 1, (1 - wy) * wx),
                    (y0 + 1, x0, wy * (1 - wx)),
                    (y0 + 1, x0 + 1, wy * wx),
                ):
                    if 0 <= yi < K and 0 <= xi < K:
                        M[r, p, yi * K + xi] += wt
    return M.astype(np.float32)


def _build():
    nc = bacc.Bacc(trn_type="TRN2")
    xs = nc.dram_tensor("xs", [BL, CIN, H, W], BF16, kind="ExternalInput")
    # wt[r, i, (j*3+kx)*O + o]: vertical-Winograd-transformed rotated filters
    wt = nc.dram_tensor("wt", [R, CIN, 12 * O], BF16, kind="ExternalInput")
    # output stays bf16 on device (the max accumulator is bf16); the host
    # widens to f32 losslessly after the gather
    y = nc.dram_tensor("y", [BL, O, H, W], BF16, kind="ExternalOutput")

    with TileContext(nc) as tc:
        with (
            tc.tile_pool(name="wpool", bufs=1) as wpool,
            tc.tile_pool(name="xpool", bufs=1) as xpool,
            tc.tile_pool(name="cpool", bufs=1) as cpool,
            tc.tile_pool(name="psum", bufs=1, space="PSUM") as ppool,
        ):
            # transformed weights: [cin, r, 12, O], all rotations resident
            wtile = wpool.tile([128, R, 12, O], BF16, name="wtile", tag="wt")

            # PE warm-up: dependency-free matmuls on a scratch tile keep the
            # PE busy from ~0.5us until the first real matmul so the HAM
            # clock gate reaches 8/8 before real work.
            dum_lhs = wpool.tile([128, 128], BF16, name="dum_lhs", tag="dum")
            nc.vector.memset(dum_lhs[:, :], 0.0)
            dum_ps = ppool.tile([128, 128], F32, name="dum_ps", tag="Pa0")
            for _ in range(60):
                nc.tensor.matmul(
                    dum_ps[:, :], dum_lhs[:, :], dum_lhs[:, :],
                    start=True, stop=True,
                )

            def load_weights(rr):
                # the sync DMA queue is serial: rotation 0 goes first, then
                # the first x block, then the remaining rotations
                for r in rr:
                    nc.sync.dma_start(out=wtile[:, r, :, :], in_=wt[r, :, :])

            load_weights([0])

            # x staging ping-pong: [34 rows, 130 cols] bf16, halo zeroed once
            xmm2 = [
                xpool.tile([128, RB + 2, W + 2], BF16, name=f"xmm{i}", tag=f"xmm{i}")
                for i in range(2)
            ]
            for i in range(2):
                # on DVE, not gpsimd: the gpsimd queue carries the x DMAs
                # and these whole-tile zeroings would delay the first load
                nc.vector.memset(xmm2[i][:, :, :], 0.0)

            # winograd row planes: [16 pairs, 130] x 4, double buffered
            tst = [
                [
                    xpool.tile([128, NPAIR, W + 2], BF16, name=f"t{p}{j}", tag=f"t{p}{j}")
                    for j in range(4)
                ]
                for p in range(2)
            ]

            def load_x(g, b, blk, chunks=None):
                h0 = blk * RB
                r0 = max(h0 - 1, 0)
                r1 = min(h0 + RB + 1, H)
                xmm = xmm2[g % 2]
                if g >= 2:
                    # restore halo-row zeros clobbered by the previous user
                    if blk == 0:
                        nc.gpsimd.memset(xmm[:, 0:1, :], 0.0)
                    elif blk == NBLK - 1:
                        nc.gpsimd.memset(xmm[:, RB + 1 : RB + 2, :], 0.0)
                d0 = r0 - (h0 - 1)
                cuts = [0, r1 - r0] if chunks is None else chunks
                for k in range(len(cuts) - 1):
                    a, c = cuts[k], cuts[k + 1]
                    # x loads ride the otherwise-idle gpsimd queue so they
                    # overlap the weight DMAs (head) and y flushes (sync)
                    nc.gpsimd.dma_start(
                        out=xmm[:, d0 + a : d0 + c, 1 : W + 1],
                        in_=xs[b, :, r0 + a : r0 + c, :],
                    )
                return xmm

            def transform(g, xmm, pair0=0, pair1=NPAIR):
                # pair s covers output rows 2s, 2s+1 of the block;
                # d_k = xmm row 2s+k (xmm row i = image row h0-1+i)
                t = tst[g % 2]
                d = [
                    xmm[:, 2 * pair0 + k : min(2 * pair1 + k, RB + 2) : 2, :]
                    for k in range(4)
                ]
                sl = slice(pair0, pair1)
                nc.vector.tensor_tensor(t[0][:, sl, :], d[0], d[2], SUB)
                nc.vector.tensor_tensor(t[1][:, sl, :], d[1], d[2], ADD)
                nc.vector.tensor_tensor(t[2][:, sl, :], d[2], d[1], SUB)
                nc.vector.tensor_tensor(t[3][:, sl, :], d[1], d[3], SUB)

            # psum: 2 phases x 2 half-tiles [2 m-planes, 4 pairs, W] f32;
            # separate tiles (tags) so the m0/m1 copy never aliases the
            # m2/m3 matmuls — PSUM dep tracking is tile-granular
            Pa = [
                ppool.tile([128, 2, 4, W], F32, name=f"Pa{p}", tag=f"Pa{p}")
                for p in range(2)
            ]
            Pb = [
                ppool.tile([128, 2, 4, W], F32, name=f"Pb{p}", tag=f"Pb{p}")
                for p in range(2)
            ]
            mba = [
                cpool.tile([128, 2, 4, W], BF16, name=f"mba{p}", tag=f"mba{p}")
                for p in range(2)
            ]
            mbb = [
                cpool.tile([128, 2, 4, W], BF16, name=f"mbb{p}", tag=f"mbb{p}")
                for p in range(2)
            ]
            uv = [
                [
                    cpool.tile([128, 4, W], BF16, name=f"uv{p}{i}", tag=f"uv{p}{i}")
                    for i in range(2)
                ]
                for p in range(2)
            ]
            # y-pair staging interleaved even/odd, so one fused max per group
            yI = [
                cpool.tile([128, 4, 2, W], BF16, name=f"yI{p}", tag=f"yI{p}")
                for p in range(2)
            ]
            # block accumulator, rows already in output order
            accI = [
                cpool.tile([128, NPAIR, 2, W], BF16, name=f"accI{p}", tag=f"accI{p}")
                for p in range(2)
            ]


            gctr = [0]

            def conv_group(g, r, sp):
                ph = gctr[0] % 2
                gctr[0] += 1
                t = tst[g % 2]

                def mms(pt, js):
                    for jj, j in enumerate(js):
                        for kx in range(3):
                            nc.tensor.matmul(
                                pt[:, jj, :, :],
                                wtile[:, r, j * 3 + kx, :],
                                t[j][:, 4 * sp : 4 * sp + 4, kx : kx + W],
                                start=(kx == 0), stop=(kx == 2),
                            )

                # copy each PSUM half as soon as its matmuls finish: the
                # m0/m1 copy overlaps the j=2,3 matmuls, and the op gating
                # the next phase's matmuls shrinks to a half-copy
                mms(Pa[ph], (0, 1))
                nc.scalar.copy(mba[ph][:, :, :, :], Pa[ph][:, :, :, :])
                mms(Pb[ph], (2, 3))
                nc.scalar.copy(mbb[ph][:, :, :, :], Pb[ph][:, :, :, :])
                m0, m1 = mba[ph][:, 0], mba[ph][:, 1]
                m2, m3 = mbb[ph][:, 0], mbb[ph][:, 1]
                u, v = uv[ph]
                acc = accI[g % 2][:, 4 * sp : 4 * sp + 4, :, :]
                yt = acc if r == 0 else yI[ph]
                nc.vector.tensor_tensor(u[:, :, :], m0, m1, ADD)
                nc.vector.tensor_tensor(v[:, :, :], m1, m2, SUB)
                nc.vector.tensor_tensor(yt[:, :, 0, :], u[:, :, :], m2, ADD)
                nc.vector.tensor_tensor(yt[:, :, 1, :], v[:, :, :], m3, SUB)
                if r > 0:
                    nc.vector.tensor_tensor(acc, acc, yt[:, :, :, :], MAX)

            def flush_block(g, b, blk, p0=0, p1=NPAIR, eng=None):
                p = g % 2
                h0 = blk * RB + 2 * p0
                (eng or nc.sync).dma_start(
                    out=y[b, :, h0 : h0 + 2 * (p1 - p0), :],
                    in_=accI[p][:, p0:p1, :, :].rearrange("i s e w -> i (s e) w"),
                )

            blocks = [(g, divmod(g, NBLK)) for g in range(BL * NBLK)]
            # first block: land the first 11 rows early so transform+matmuls
            # for the leading pairs start before the whole block arrives
            xmm0 = load_x(0, *blocks[0][1], chunks=[0, 10, 33])
            load_weights(range(1, R))
            transform(0, xmm0, 0, 4)
            transform(0, xmm0, 4, NPAIR)
            for g, (b, blk) in blocks[:-1]:
                for r in range(R):
                    if r == 1 and g + 1 < len(blocks):
                        nb, nblk = blocks[g + 1][1]
                        transform(g + 1, load_x(g + 1, nb, nblk))
                    if r == 2 and g > 0:
                        flush_block(g - 1, *blocks[g - 1][1])
                    for sp in range(NG):
                        conv_group(g, r, sp)
            # last block runs pair-group-major: each 8-row strip finishes all
            # rotations ~21us before the end, so its flush + output DMA hide
            # under the remaining strips' matmuls instead of trailing them
            g, (b, blk) = blocks[-1]
            for sp in range(NG):
                for r in range(R):
                    conv_group(g, r, sp)
                    if sp == 0 and r == 2:
                        flush_block(g - 1, *blocks[g - 1][1])
                # alternate queues so the two sliver transfers overlap
                flush_block(g, b, blk, 4 * sp, 4 * sp + 2,
                            eng=nc.sync if sp % 2 == 0 else nc.gpsimd)
                flush_block(g, b, blk, 4 * sp + 2, 4 * sp + 4,
                            eng=nc.gpsimd if sp % 2 == 0 else nc.sync)
    nc.finalize()
    return nc


def _get_nc():
    if "wino" not in _NC_CACHE:
        _NC_CACHE["wino"] = _build()
    return _NC_CACHE["wino"]


def _prep_weights(weight, rot_alpha):
    """Rotate the filter bank by the 8 angles and fold the vertical Winograd
    F(2,3) G-transform in; returns [R, CIN, 12*O] bf16."""
    M = _rot_mats(rot_alpha)
    w_r = (
        weight.reshape(O, R, CIN, 9).transpose(1, 0, 2, 3).astype(np.float64)
    )  # (R, O, I, 9)
    rot = np.einsum("rpq,roiq->roip", M.astype(np.float64), w_r)
    rot = rot.reshape(R, O, CIN, 3, 3)  # (ky, kx)
    G = np.array(
        [[1, 0, 0], [0.5, 0.5, 0.5], [0.5, -0.5, 0.5], [0, 0, 1]], np.float64
    )
    gp = np.einsum("jk,roikx->rijxo", G, rot)  # (R, I, 4, 3, O)
    return np.ascontiguousarray(
        gp.reshape(R, CIN, 12 * O).astype(np.float32).astype(BF16NP)
    )


def kernel(x, weight, rot_alpha):
    global LAST_RESULTS
    x = np.asarray(x, np.float32)
    weight = np.asarray(weight, np.float32)
    rot_alpha = np.asarray(rot_alpha, np.float32)

    wt = _prep_weights(weight, rot_alpha)
    xb = np.ascontiguousarray(x.astype(BF16NP))

    nc = _get_nc()
    in_maps = [
        {"xs": np.ascontiguousarray(xb[c * BL : (c + 1) * BL]), "wt": wt}
        for c in range(NCORES)
    ]
    try:
        res = run_bass_kernel_spmd(nc, in_maps, list(range(NCORES)), trace=_TRACE)
    except Exception:
        # One retry (without tracing): a failed compile or an aborted run can
        # leave a NeuronCore transiently wedged; the next attempt recovers.
        res = run_bass_kernel_spmd(nc, in_maps, list(range(NCORES)), trace=False)
    LAST_RESULTS = res
    return np.concatenate(
        [res.results[c]["y"] for c in range(NCORES)], axis=0
    ).astype(np.float32)


# revision 38
# speedup vs baseline: 1.0049x; 1.0043x over previous
"""Equivariant rotation conv for Trainium2, 8-core batch-parallel,
vertical-Winograd F(2,3) formulation.

Computes: rotate a (128*8, 128, 3, 3) filter bank by 8 data-dependent angles
(bilinear resampling), run a 3x3 same-padded conv of x (16,128,128,128) with
all 8*128 rotated filters, then max over the 8 rotations -> (16,128,128,128).

Sharding: data-parallel over batch, 2 images per core; the filter bank is
replicated.  The rotation (a 9x9 bilinear mix, a pure function of the 8
rot_alpha scalars) and a vertical Winograd F(2,3) G-transform are folded into
the weights on the host, producing 4 transformed vertical taps x 3 horizontal
taps per rotation in bf16.  On device, per core:
  - x arrives pre-cast to bf16; per 32-row block the DVE builds 4 transformed
    row-planes (t0 = d0-d2, t1 = d1+d2, t2 = d2-d1, t3 = d1-d3 over row pairs)
    with strided-row tensor_tensor ops in the 2x bf16 mode,
  - the conv needs only 12 PE matmuls per 8 output rows (4 m-planes x 3
    horizontal taps, f32 PSUM accumulation) instead of 18 direct ones: the
    two output rows of each pair are recombined as y0 = m0+m1+m2,
    y1 = m1-m2-m3 outside the PE,
  - ACT copies each PSUM half-group (2 banks) to bf16 SBUF as soon as its 6
    matmuls land -- PSUM dep tracking is tile-granular, so the halves live
    in separate tiles (Pa/Pb) to keep the m0/m1 copy off the m2/m3 matmuls'
    critical path,
  - the DVE runs the inverse transform + running rotation max in the 2x bf16
    mode (u = m0+m1, v = m1-m2, y pair into an interleaved even/odd tile,
    one fused 8-row max),
  - the interleaved bf16 accumulator (rows already in output order) is
    DMA'd out directly as bf16 and widened to f32 on the host (lossless);
    the final block runs pair-group-major so its flush DMAs hide under the
    remaining matmuls.

Measured on trn2 (8 cores): ~697 us vs ~1030 us for the direct 9-tap bf16
kernel (PE ~97% busy, zero >300ns pipeline gaps, at the 512-col matmul
streaming roofline; DVE ~79%, ACT ~74%).  Numerics: rel_l2 ~3.7e-3 vs the
f32 reference (bf16 products, f32 PSUM accumulation, bf16 max tree; gate
is 2e-2).
"""

import numpy as np
import ml_dtypes


def _install_axon_hooks_shim():
    """Provide antenv.axon_hooks (NTFF profile hook) when the image's antenv
    lacks it, so run_bass_kernel_spmd(trace=True) works instead of crashing
    on import."""
    import contextlib
    import ctypes
    import os
    import sys
    import types

    try:
        import antenv.axon_hooks  # noqa: F401

        return
    except ImportError:
        pass

    state = {"hook": None, "resolved": False}

    def _make_hook():
        so_path = os.environ.get("AXON_PJRT_SO", "/opt/axon/libaxon_pjrt.so")
        if not os.path.exists(so_path):
            return None
        lib = ctypes.CDLL(so_path)
        if not hasattr(lib, "axon_start_nrt_profile"):
            return None
        lib.axon_start_nrt_profile.argtypes = [
            ctypes.POINTER(ctypes.c_int64),
            ctypes.c_size_t,
        ]
        lib.axon_start_nrt_profile.restype = ctypes.c_int64
        lib.axon_stop_nrt_profile.argtypes = [ctypes.c_char_p]
        lib.axon_stop_nrt_profile.restype = ctypes.c_int64

        @contextlib.contextmanager
        def _hook(output_dir, device_ids):
            import jax

            jax.devices()
            if device_ids:
                ids = (ctypes.c_int64 * len(device_ids))(*device_ids)
                rc = lib.axon_start_nrt_profile(ids, len(device_ids))
            else:
                rc = lib.axon_start_nrt_profile(None, 0)
            if rc != 0:
                raise RuntimeError(f"axon_start_nrt_profile rc={rc}")
            try:
                yield
            finally:
                n = lib.axon_stop_nrt_profile(str(output_dir).encode())
                if n < 0:
                    raise RuntimeError(f"axon_stop_nrt_profile rc={n}")
                print(f"profile: {n} file(s) written to {output_dir}")

        return _hook

    mod = types.ModuleType("antenv.axon_hooks")

    def set_axon_ntff_profile_hook(h):
        state["hook"] = h
        state["resolved"] = True

    def get_axon_ntff_profile_hook():
        if not state["resolved"]:
            state["hook"] = _make_hook()
            state["resolved"] = True
        return state["hook"]

    mod.set_axon_ntff_profile_hook = set_axon_ntff_profile_hook
    mod.get_axon_ntff_profile_hook = get_axon_ntff_profile_hook
    sys.modules["antenv.axon_hooks"] = mod


_install_axon_hooks_shim()

import concourse.bass as bass  # noqa: E402,F401
import concourse.mybir as mybir  # noqa: E402
from concourse import bacc  # noqa: E402
from concourse.bass_utils import run_bass_kernel_spmd  # noqa: E402
from concourse.tile import TileContext  # noqa: E402

F32 = mybir.dt.float32
BF16 = mybir.dt.bfloat16
BF16NP = ml_dtypes.bfloat16

B, CIN, H, W = 16, 128, 128, 128
R, O, K = 8, 128, 3
NCORES = 8
BL = B // NCORES   # images per core
RB = 32            # output rows per block
NPAIR = RB // 2    # winograd row pairs per block
NG = NPAIR // 4    # matmul groups (4 pairs = 8 output rows) per block
NBLK = H // RB

ADD = mybir.AluOpType.add
SUB = mybir.AluOpType.subtract
MAX = mybir.AluOpType.max

_TRACE = False
LAST_RESULTS = None
_NC_CACHE = {}


def _rot_mats(rot_alpha):
    """Per-rotation 9x9 bilinear resampling matrices, matching the reference
    F.grid_sample(align_corners=True, zeros) tap logic exactly.

    M[r, p, q]: coefficient of original tap q = (qy*3+qx) in rotated tap
    p = (py*3+px)."""
    M = np.zeros((R, 9, 9), np.float64)
    lin = np.linspace(-1.0, 1.0, K)
    for r in range(R):
        ang = float(rot_alpha[r]) * (np.pi / 4.0) * r
        c, s = np.cos(ang), np.sin(ang)
        for a in range(K):          # output row (gy = lin[a])
            for b in range(K):      # output col (gx = lin[b])
                gx, gy = lin[b], lin[a]
                xs = c * gx - s * gy
                ys = s * gx + c * gy
                ix = (xs + 1.0) * 0.5 * (K - 1)
                iy = (ys + 1.0) * 0.5 * (K - 1)
                x0 = int(np.floor(ix))
                y0 = int(np.floor(iy))
                wx, wy = ix - x0, iy - y0
                p = a * K + b
                for yi, xi, wt in (
                    (y0, x0, (1 - wy) * (1 - wx)),
                    (y0, x0 + 1, (1 - wy) * wx),
                    (y0 + 1, x0, wy * (1 - wx)),
                    (y0 + 1, x0 + 1, wy * wx),
                ):
                    if 0 <= yi < K and 0 <= xi < K:
                        M[r, p, yi * K + xi] += wt
    return M.astype(np.float32)


def _build():
    nc = bacc.Bacc(trn_type="TRN2")
    xs = nc.dram_tensor("xs", [BL, CIN, H, W], BF16, kind="ExternalInput")
    # wt[r, i, (j*3+kx)*O + o]: vertical-Winograd-transformed rotated filters
    wt = nc.dram_tensor("wt", [R, CIN, 12 * O], BF16, kind="ExternalInput")
    # output stays bf16 on device (the max accumulator is bf16); the host
    # widens to f32 losslessly after the gather
    y = nc.dram_tensor("y", [BL, O, H, W], BF16, kind="ExternalOutput")

    with TileContext(nc) as tc:
        with (
            tc.tile_pool(name="wpool", bufs=1) as wpool,
            tc.tile_pool(name="xpool", bufs=1) as xpool,
            tc.tile_pool(name="cpool", bufs=1) as cpool,
            tc.tile_pool(name="psum", bufs=1, space="PSUM") as ppool,
        ):
            # transformed weights: [cin, r, 12, O], all rotations resident
            wtile = wpool.tile([128, R, 12, O], BF16, name="wtile", tag="wt")

            # PE warm-up: dependency-free matmuls on a scratch tile keep the
            # PE busy from ~0.5us until the first real matmul so the HAM
            # clock gate reaches 8/8 before real work.
            dum_lhs = wpool.tile([128, 128], BF16, name="dum_lhs", tag="dum")
            nc.vector.memset(dum_lhs[:, :], 0.0)
            dum_ps = ppool.tile([128, 128], F32, name="dum_ps", tag="Pa0")
            for _ in range(200):
                nc.tensor.matmul(
                    dum_ps[:, :], dum_lhs[:, :], dum_lhs[:, :],
                    start=True, stop=True,
                )

            def load_weights(rr):
                # the sync DMA queue is serial: rotation 0 goes first, then
                # the first x block, then the remaining rotations
                for r in rr:
                    nc.sync.dma_start(out=wtile[:, r, :, :], in_=wt[r, :, :])

            load_weights([0])

            # x staging ping-pong: [34 rows, 130 cols] bf16, halo zeroed once
            xmm2 = [
                xpool.tile([128, RB + 2, W + 2], BF16, name=f"xmm{i}", tag=f"xmm{i}")
                for i in range(2)
            ]
            for i in range(2):
                # on DVE, not gpsimd: the gpsimd queue carries the x DMAs
                # and these whole-tile zeroings would delay the first load
                nc.vector.memset(xmm2[i][:, :, :], 0.0)

            # winograd row planes: [16 pairs, 130] x 4, double buffered
            tst = [
                [
                    xpool.tile([128, NPAIR, W + 2], BF16, name=f"t{p}{j}", tag=f"t{p}{j}")
                    for j in range(4)
                ]
                for p in range(2)
            ]

            def load_x(g, b, blk, chunks=None):
                h0 = blk * RB
                r0 = max(h0 - 1, 0)
                r1 = min(h0 + RB + 1, H)
                xmm = xmm2[g % 2]
                if g >= 2:
                    # restore halo-row zeros clobbered by the previous user
                    if blk == 0:
                        nc.gpsimd.memset(xmm[:, 0:1, :], 0.0)
                    elif blk == NBLK - 1:
                        nc.gpsimd.memset(xmm[:, RB + 1 : RB + 2, :], 0.0)
                d0 = r0 - (h0 - 1)
                cuts = [0, r1 - r0] if chunks is None else chunks
                for k in range(len(cuts) - 1):
                    a, c = cuts[k], cuts[k + 1]
                    nc.sync.dma_start(
                        out=xmm[:, d0 + a : d0 + c, 1 : W + 1],
                        in_=xs[b, :, r0 + a : r0 + c, :],
                    )
                return xmm

            def transform(g, xmm, pair0=0, pair1=NPAIR):
                # pair s covers output rows 2s, 2s+1 of the block;
                # d_k = xmm row 2s+k (xmm row i = image row h0-1+i)
                t = tst[g % 2]
                d = [
                    xmm[:, 2 * pair0 + k : min(2 * pair1 + k, RB + 2) : 2, :]
                    for k in range(4)
                ]
                sl = slice(pair0, pair1)
                nc.vector.tensor_tensor(t[0][:, sl, :], d[0], d[2], SUB)
                nc.vector.tensor_tensor(t[1][:, sl, :], d[1], d[2], ADD)
                nc.vector.tensor_tensor(t[2][:, sl, :], d[2], d[1], SUB)
                nc.vector.tensor_tensor(t[3][:, sl, :], d[1], d[3], SUB)

            # psum: 2 phases x 2 half-tiles [2 m-planes, 4 pairs, W] f32;
            # separate tiles (tags) so the m0/m1 copy never aliases the
            # m2/m3 matmuls — PSUM dep tracking is tile-granular
            Pa = [
                ppool.tile([128, 2, 4, W], F32, name=f"Pa{p}", tag=f"Pa{p}")
                for p in range(2)
            ]
            Pb = [
                ppool.tile([128, 2, 4, W], F32, name=f"Pb{p}", tag=f"Pb{p}")
                for p in range(2)
            ]
            mba = [
                cpool.tile([128, 2, 4, W], BF16, name=f"mba{p}", tag=f"mba{p}")
                for p in range(2)
            ]
            mbb = [
                cpool.tile([128, 2, 4, W], BF16, name=f"mbb{p}", tag=f"mbb{p}")
                for p in range(2)
            ]
            uv = [
                [
                    cpool.tile([128, 4, W], BF16, name=f"uv{p}{i}", tag=f"uv{p}{i}")
                    for i in range(2)
                ]
                for p in range(2)
            ]
            # y-pair staging interleaved even/odd, so one fused max per group
            yI = [
                cpool.tile([128, 4, 2, W], BF16, name=f"yI{p}", tag=f"yI{p}")
                for p in range(2)
            ]
            # block accumulator, rows already in output order
            accI = [
                cpool.tile([128, NPAIR, 2, W], BF16, name=f"accI{p}", tag=f"accI{p}")
                for p in range(2)
            ]


            gctr = [0]

            def conv_group(g, r, sp):
                ph = gctr[0] % 2
                gctr[0] += 1
                t = tst[g % 2]

                def mms(pt, js):
                    for jj, j in enumerate(js):
                        for kx in range(3):
                            nc.tensor.matmul(
                                pt[:, jj, :, :],
                                wtile[:, r, j * 3 + kx, :],
                                t[j][:, 4 * sp : 4 * sp + 4, kx : kx + W],
                                start=(kx == 0), stop=(kx == 2),
                            )

                # copy each PSUM half as soon as its matmuls finish: the
                # m0/m1 copy overlaps the j=2,3 matmuls, and the op gating
                # the next phase's matmuls shrinks to a half-copy
                mms(Pa[ph], (0, 1))
                nc.scalar.copy(mba[ph][:, :, :, :], Pa[ph][:, :, :, :])
                mms(Pb[ph], (2, 3))
                nc.scalar.copy(mbb[ph][:, :, :, :], Pb[ph][:, :, :, :])
                m0, m1 = mba[ph][:, 0], mba[ph][:, 1]
                m2, m3 = mbb[ph][:, 0], mbb[ph][:, 1]
                u, v = uv[ph]
                acc = accI[g % 2][:, 4 * sp : 4 * sp + 4, :, :]
                yt = acc if r == 0 else yI[ph]
                nc.vector.tensor_tensor(u[:, :, :], m0, m1, ADD)
                nc.vector.tensor_tensor(v[:, :, :], m1, m2, SUB)
                nc.vector.tensor_tensor(yt[:, :, 0, :], u[:, :, :], m2, ADD)
                nc.vector.tensor_tensor(yt[:, :, 1, :], v[:, :, :], m3, SUB)
                if r > 0:
                    nc.vector.tensor_tensor(acc, acc, yt[:, :, :, :], MAX)

            def flush_block(g, b, blk, p0=0, p1=NPAIR, eng=None):
                p = g % 2
                h0 = blk * RB + 2 * p0
                (eng or nc.sync).dma_start(
                    out=y[b, :, h0 : h0 + 2 * (p1 - p0), :],
                    in_=accI[p][:, p0:p1, :, :].rearrange("i s e w -> i (s e) w"),
                )

            blocks = [(g, divmod(g, NBLK)) for g in range(BL * NBLK)]
            # first block: land the first 11 rows early so transform+matmuls
            # for the leading pairs start before the whole block arrives
            xmm0 = load_x(0, *blocks[0][1], chunks=[0, 10, 33])
            load_weights(range(1, R))
            transform(0, xmm0, 0, 4)
            transform(0, xmm0, 4, NPAIR)
            for g, (b, blk) in blocks[:-1]:
                for r in range(R):
                    if r == 1 and g + 1 < len(blocks):
                        nb, nblk = blocks[g + 1][1]
                        transform(g + 1, load_x(g + 1, nb, nblk))
                    if r == 2 and g > 0:
                        flush_block(g - 1, *blocks[g - 1][1])
                    for sp in range(NG):
                        conv_group(g, r, sp)
            # last block runs pair-group-major: each 8-row strip finishes all
            # rotations ~21us before the end, so its flush + output DMA hide
            # under the remaining strips' matmuls instead of trailing them
            g, (b, blk) = blocks[-1]
            for sp in range(NG):
                for r in range(R):
                    conv_group(g, r, sp)
                    if sp == 0 and r == 2:
                        flush_block(g - 1, *blocks[g - 1][1])
                # alternate queues so the two sliver transfers overlap
                flush_block(g, b, blk, 4 * sp, 4 * sp + 2,
                            eng=nc.sync if sp % 2 == 0 else nc.gpsimd)
                flush_block(g, b, blk, 4 * sp + 2, 4 * sp + 4,
                            eng=nc.gpsimd if sp % 2 == 0 else nc.sync)
    nc.finalize()
    return nc


def _get_nc():
    if "wino" not in _NC_CACHE:
        _NC_CACHE["wino"] = _build()
    return _NC_CACHE["wino"]


def _prep_weights(weight, rot_alpha):
    """Rotate the filter bank by the 8 angles and fold the vertical Winograd
    F(2,3) G-transform in; returns [R, CIN, 12*O] bf16."""
    M = _rot_mats(rot_alpha)
    w_r = (
        weight.reshape(O, R, CIN, 9).transpose(1, 0, 2, 3).astype(np.float64)
    )  # (R, O, I, 9)
    rot = np.einsum("rpq,roiq->roip", M.astype(np.float64), w_r)
    rot = rot.reshape(R, O, CIN, 3, 3)  # (ky, kx)
    G = np.array(
        [[1, 0, 0], [0.5, 0.5, 0.5], [0.5, -0.5, 0.5], [0, 0, 1]], np.float64
    )
    gp = np.einsum("jk,roikx->rijxo", G, rot)  # (R, I, 4, 3, O)
    return np.ascontiguousarray(
        gp.reshape(R, CIN, 12 * O).astype(np.float32).astype(BF16NP)
    )


def kernel(x, weight, rot_alpha):
    global LAST_RESULTS
    x = np.asarray(x, np.float32)
    weight = np.asarray(weight, np.float32)
    rot_alpha = np.asarray(rot_alpha, np.float32)

    wt = _prep_weights(weight, rot_alpha)
    xb = np.ascontiguousarray(x.astype(BF16NP))

    nc = _get_nc()
    in_maps = [
        {"xs": np.ascontiguousarray(xb[c * BL : (c + 1) * BL]), "wt": wt}
        for c in range(NCORES)
    ]
    try:
        res = run_bass_kernel_spmd(nc, in_maps, list(range(NCORES)), trace=_TRACE)
    except Exception:
        # One retry (without tracing): a failed compile or an aborted run can
        # leave a NeuronCore transiently wedged; the next attempt recovers.
        res = run_bass_kernel_spmd(nc, in_maps, list(range(NCORES)), trace=False)
    LAST_RESULTS = res
    return np.concatenate(
        [res.results[c]["y"] for c in range(NCORES)], axis=0
    ).astype(np.float32)


# revision 40
# speedup vs baseline: 1.0142x; 1.0093x over previous
"""Equivariant rotation conv for Trainium2, 8-core batch-parallel,
vertical-Winograd F(2,3) formulation.

Computes: rotate a (128*8, 128, 3, 3) filter bank by 8 data-dependent angles
(bilinear resampling), run a 3x3 same-padded conv of x (16,128,128,128) with
all 8*128 rotated filters, then max over the 8 rotations -> (16,128,128,128).

Sharding: data-parallel over batch, 2 images per core; the filter bank is
replicated.  The rotation (a 9x9 bilinear mix, a pure function of the 8
rot_alpha scalars) and a vertical Winograd F(2,3) G-transform are folded into
the weights on the host, producing 4 transformed vertical taps x 3 horizontal
taps per rotation in bf16.  On device, per core:
  - x arrives pre-cast to bf16; per 32-row block the DVE builds 4 transformed
    row-planes (t0 = d0-d2, t1 = d1+d2, t2 = d2-d1, t3 = d1-d3 over row pairs)
    with strided-row tensor_tensor ops in the 2x bf16 mode,
  - the conv needs only 12 PE matmuls per 8 output rows (4 m-planes x 3
    horizontal taps, f32 PSUM accumulation) instead of 18 direct ones: the
    two output rows of each pair are recombined as y0 = m0+m1+m2,
    y1 = m1-m2-m3 outside the PE,
  - ACT copies each PSUM half-group (2 banks) to bf16 SBUF as soon as its 6
    matmuls land -- PSUM dep tracking is tile-granular, so the halves live
    in separate tiles (Pa/Pb) to keep the m0/m1 copy off the m2/m3 matmuls'
    critical path,
  - the DVE runs the inverse transform + running rotation max in the 2x bf16
    mode (u = m0+m1, v = m1-m2, y pair into an interleaved even/odd tile,
    one fused 8-row max),
  - the interleaved bf16 accumulator (rows already in output order) is
    DMA'd out directly as bf16 and widened to f32 on the host (lossless);
    the final block runs pair-group-major so its flush DMAs hide under the
    remaining matmuls.

Measured on trn2 (8 cores): ~697 us vs ~1030 us for the direct 9-tap bf16
kernel (PE ~97% busy, zero >300ns pipeline gaps, at the 512-col matmul
streaming roofline; DVE ~79%, ACT ~74%).  Numerics: rel_l2 ~3.7e-3 vs the
f32 reference (bf16 products, f32 PSUM accumulation, bf16 max tree; gate
is 2e-2).
"""

import numpy as np
import ml_dtypes


def _install_axon_hooks_shim():
    """Provide antenv.axon_hooks (NTFF profile hook) when the image's antenv
    lacks it, so run_bass_kernel_spmd(trace=True) works instead of crashing
    on import."""
    import contextlib
    import ctypes
    import os
    import sys
    import types

    try:
        import antenv.axon_hooks  # noqa: F401

        return
    except ImportError:
        pass

    state = {"hook": None, "resolved": False}

    def _make_hook():
        so_path = os.environ.get("AXON_PJRT_SO", "/opt/axon/libaxon_pjrt.so")
        if not os.path.exists(so_path):
            return None
        lib = ctypes.CDLL(so_path)
        if not hasattr(lib, "axon_start_nrt_profile"):
            return None
        lib.axon_start_nrt_profile.argtypes = [
            ctypes.POINTER(ctypes.c_int64),
            ctypes.c_size_t,
        ]
        lib.axon_start_nrt_profile.restype = ctypes.c_int64
        lib.axon_stop_nrt_profile.argtypes = [ctypes.c_char_p]
        lib.axon_stop_nrt_profile.restype = ctypes.c_int64

        @contextlib.contextmanager
        def _hook(output_dir, device_ids):
            import jax

            jax.devices()
            if device_ids:
                ids = (ctypes.c_int64 * len(device_ids))(*device_ids)
                rc = lib.axon_start_nrt_profile(ids, len(device_ids))
            else:
                rc = lib.axon_start_nrt_profile(None, 0)
            if rc != 0:
                raise RuntimeError(f"axon_start_nrt_profile rc={rc}")
            try:
                yield
            finally:
                n = lib.axon_stop_nrt_profile(str(output_dir).encode())
                if n < 0:
                    raise RuntimeError(f"axon_stop_nrt_profile rc={n}")
                print(f"profile: {n} file(s) written to {output_dir}")

        return _hook

    mod = types.ModuleType("antenv.axon_hooks")

    def set_axon_ntff_profile_hook(h):
        state["hook"] = h
        state["resolved"] = True

    def get_axon_ntff_profile_hook():
        if not state["resolved"]:
            state["hook"] = _make_hook()
            state["resolved"] = True
        return state["hook"]

    mod.set_axon_ntff_profile_hook = set_axon_ntff_profile_hook
    mod.get_axon_ntff_profile_hook = get_axon_ntff_profile_hook
    sys.modules["antenv.axon_hooks"] = mod


_install_axon_hooks_shim()

import concourse.bass as bass  # noqa: E402,F401
import concourse.mybir as mybir  # noqa: E402
from concourse import bacc  # noqa: E402
from concourse.bass_utils import run_bass_kernel_spmd  # noqa: E402
from concourse.tile import TileContext  # noqa: E402

F32 = mybir.dt.float32
BF16 = mybir.dt.bfloat16
BF16NP = ml_dtypes.bfloat16

B, CIN, H, W = 16, 128, 128, 128
R, O, K = 8, 128, 3
NCORES = 8
BL = B // NCORES   # images per core
RB = 32            # output rows per block
NPAIR = RB // 2    # winograd row pairs per block
NG = NPAIR // 4    # matmul groups (4 pairs = 8 output rows) per block
NBLK = H // RB

ADD = mybir.AluOpType.add
SUB = mybir.AluOpType.subtract
MAX = mybir.AluOpType.max

_TRACE = False
LAST_RESULTS = None
_NC_CACHE = {}


def _rot_mats(rot_alpha):
    """Per-rotation 9x9 bilinear resampling matrices, matching the reference
    F.grid_sample(align_corners=True, zeros) tap logic exactly.

    M[r, p, q]: coefficient of original tap q = (qy*3+qx) in rotated tap
    p = (py*3+px)."""
    M = np.zeros((R, 9, 9), np.float64)
    lin = np.linspace(-1.0, 1.0, K)
    for r in range(R):
        ang = float(rot_alpha[r]) * (np.pi / 4.0) * r
        c, s = np.cos(ang), np.sin(ang)
        for a in range(K):          # output row (gy = lin[a])
            for b in range(K):      # output col (gx = lin[b])
                gx, gy = lin[b], lin[a]
                xs = c * gx - s * gy
                ys = s * gx + c * gy
                ix = (xs + 1.0) * 0.5 * (K - 1)
                iy = (ys + 1.0) * 0.5 * (K - 1)
                x0 = int(np.floor(ix))
                y0 = int(np.floor(iy))
                wx, wy = ix - x0, iy - y0
                p = a * K + b
                for yi, xi, wt in (
                    (y0, x0, (1 - wy) * (1 - wx)),
                    (y0, x0 + 1, (1 - wy) * wx),
                    (y0 + 1, x0, wy * (1 - wx)),
                    (y0 + 1, x0 + 1, wy * wx),
                ):
                    if 0 <= yi < K and 0 <= xi < K:
                        M[r, p, yi * K + xi] += wt
    return M.astype(np.float32)


def _build():
    nc = bacc.Bacc(trn_type="TRN2")
    xs = nc.dram_tensor("xs", [BL, CIN, H, W], BF16, kind="ExternalInput")
    # wt[r, i, (j*3+kx)*O + o]: vertical-Winograd-transformed rotated filters
    wt = nc.dram_tensor("wt", [R, CIN, 12 * O], BF16, kind="ExternalInput")
    # output stays bf16 on device (the max accumulator is bf16); the host
    # widens to f32 losslessly after the gather
    y = nc.dram_tensor("y", [BL, O, H, W], BF16, kind="ExternalOutput")

    with TileContext(nc) as tc:
        with (
            tc.tile_pool(name="wpool", bufs=1) as wpool,
            tc.tile_pool(name="xpool", bufs=1) as xpool,
            tc.tile_pool(name="cpool", bufs=1) as cpool,
            tc.tile_pool(name="psum", bufs=1, space="PSUM") as ppool,
        ):
            # transformed weights: [cin, r, 12, O], all rotations resident
            wtile = wpool.tile([128, R, 12, O], BF16, name="wtile", tag="wt")

            # PE warm-up: dependency-free matmuls on a scratch tile keep the
            # PE busy from ~0.5us until the first real matmul so the HAM
            # clock gate reaches 8/8 before real work.
            dum_lhs = wpool.tile([128, 128], BF16, name="dum_lhs", tag="dum")
            nc.vector.memset(dum_lhs[:, :], 0.0)
            dum_ps = ppool.tile([128, 128], F32, name="dum_ps", tag="Pa0")
            for _ in range(110):
                nc.tensor.matmul(
                    dum_ps[:, :], dum_lhs[:, :], dum_lhs[:, :],
                    start=True, stop=True,
                )

            def load_weights(rr):
                # the sync DMA queue is serial: rotation 0 goes first, then
                # the first x block, then the remaining rotations
                for r in rr:
                    nc.sync.dma_start(out=wtile[:, r, :, :], in_=wt[r, :, :])

            load_weights([0])

            # x staging ping-pong: [34 rows, 130 cols] bf16, halo zeroed once
            xmm2 = [
                xpool.tile([128, RB + 2, W + 2], BF16, name=f"xmm{i}", tag=f"xmm{i}")
                for i in range(2)
            ]
            # zero only the halo (the block DMAs rewrite the interior every
            # use): a whole-tile memset costs ~3.7us on DVE and gates the
            # first x DMA via WAW, delaying the pipeline start
            for i in range(2):
                nc.gpsimd.memset(xmm2[i][:, :, 0:1], 0.0)
                nc.gpsimd.memset(xmm2[i][:, :, W + 1 : W + 2], 0.0)
            nc.gpsimd.memset(xmm2[0][:, 0:1, :], 0.0)

            # winograd row planes: [16 pairs, 130] x 4, double buffered
            tst = [
                [
                    xpool.tile([128, NPAIR, W + 2], BF16, name=f"t{p}{j}", tag=f"t{p}{j}")
                    for j in range(4)
                ]
                for p in range(2)
            ]

            def load_x(g, b, blk, chunks=None):
                h0 = blk * RB
                r0 = max(h0 - 1, 0)
                r1 = min(h0 + RB + 1, H)
                xmm = xmm2[g % 2]
                if g >= 2:
                    # restore halo-row zeros clobbered by the previous user
                    if blk == 0:
                        nc.gpsimd.memset(xmm[:, 0:1, :], 0.0)
                    elif blk == NBLK - 1:
                        nc.gpsimd.memset(xmm[:, RB + 1 : RB + 2, :], 0.0)
                d0 = r0 - (h0 - 1)
                cuts = [0, r1 - r0] if chunks is None else chunks
                for k in range(len(cuts) - 1):
                    a, c = cuts[k], cuts[k + 1]
                    nc.sync.dma_start(
                        out=xmm[:, d0 + a : d0 + c, 1 : W + 1],
                        in_=xs[b, :, r0 + a : r0 + c, :],
                    )
                return xmm

            def transform(g, xmm, pair0=0, pair1=NPAIR):
                # pair s covers output rows 2s, 2s+1 of the block;
                # d_k = xmm row 2s+k (xmm row i = image row h0-1+i)
                t = tst[g % 2]
                d = [
                    xmm[:, 2 * pair0 + k : min(2 * pair1 + k, RB + 2) : 2, :]
                    for k in range(4)
                ]
                sl = slice(pair0, pair1)
                nc.vector.tensor_tensor(t[0][:, sl, :], d[0], d[2], SUB)
                nc.vector.tensor_tensor(t[1][:, sl, :], d[1], d[2], ADD)
                nc.vector.tensor_tensor(t[2][:, sl, :], d[2], d[1], SUB)
                nc.vector.tensor_tensor(t[3][:, sl, :], d[1], d[3], SUB)

            # psum: 2 phases x 2 half-tiles [2 m-planes, 4 pairs, W] f32;
            # separate tiles (tags) so the m0/m1 copy never aliases the
            # m2/m3 matmuls — PSUM dep tracking is tile-granular
            Pa = [
                ppool.tile([128, 2, 4, W], F32, name=f"Pa{p}", tag=f"Pa{p}")
                for p in range(2)
            ]
            Pb = [
                ppool.tile([128, 2, 4, W], F32, name=f"Pb{p}", tag=f"Pb{p}")
                for p in range(2)
            ]
            mba = [
                cpool.tile([128, 2, 4, W], BF16, name=f"mba{p}", tag=f"mba{p}")
                for p in range(2)
            ]
            mbb = [
                cpool.tile([128, 2, 4, W], BF16, name=f"mbb{p}", tag=f"mbb{p}")
                for p in range(2)
            ]
            uv = [
                [
                    cpool.tile([128, 4, W], BF16, name=f"uv{p}{i}", tag=f"uv{p}{i}")
                    for i in range(2)
                ]
                for p in range(2)
            ]
            # y-pair staging interleaved even/odd, so one fused max per group
            yI = [
                cpool.tile([128, 4, 2, W], BF16, name=f"yI{p}", tag=f"yI{p}")
                for p in range(2)
            ]
            # block accumulator, rows already in output order
            accI = [
                cpool.tile([128, NPAIR, 2, W], BF16, name=f"accI{p}", tag=f"accI{p}")
                for p in range(2)
            ]


            gctr = [0]

            def conv_group(g, r, sp):
                ph = gctr[0] % 2
                gctr[0] += 1
                t = tst[g % 2]

                def mms(pt, js):
                    for jj, j in enumerate(js):
                        for kx in range(3):
                            nc.tensor.matmul(
                                pt[:, jj, :, :],
                                wtile[:, r, j * 3 + kx, :],
                                t[j][:, 4 * sp : 4 * sp + 4, kx : kx + W],
                                start=(kx == 0), stop=(kx == 2),
                            )

                # copy each PSUM half as soon as its matmuls finish: the
                # m0/m1 copy overlaps the j=2,3 matmuls, and the op gating
                # the next phase's matmuls shrinks to a half-copy
                mms(Pa[ph], (0, 1))
                nc.scalar.copy(mba[ph][:, :, :, :], Pa[ph][:, :, :, :])
                mms(Pb[ph], (2, 3))
                nc.scalar.copy(mbb[ph][:, :, :, :], Pb[ph][:, :, :, :])
                m0, m1 = mba[ph][:, 0], mba[ph][:, 1]
                m2, m3 = mbb[ph][:, 0], mbb[ph][:, 1]
                u, v = uv[ph]
                acc = accI[g % 2][:, 4 * sp : 4 * sp + 4, :, :]
                yt = acc if r == 0 else yI[ph]
                nc.vector.tensor_tensor(u[:, :, :], m0, m1, ADD)
                nc.vector.tensor_tensor(v[:, :, :], m1, m2, SUB)
                nc.vector.tensor_tensor(yt[:, :, 0, :], u[:, :, :], m2, ADD)
                nc.vector.tensor_tensor(yt[:, :, 1, :], v[:, :, :], m3, SUB)
                if r > 0:
                    nc.vector.tensor_tensor(acc, acc, yt[:, :, :, :], MAX)

            def flush_block(g, b, blk, p0=0, p1=NPAIR, eng=None):
                p = g % 2
                h0 = blk * RB + 2 * p0
                (eng or nc.sync).dma_start(
                    out=y[b, :, h0 : h0 + 2 * (p1 - p0), :],
                    in_=accI[p][:, p0:p1, :, :].rearrange("i s e w -> i (s e) w"),
                )

            blocks = [(g, divmod(g, NBLK)) for g in range(BL * NBLK)]
            # first block: land the first 11 rows early so transform+matmuls
            # for the leading pairs start before the whole block arrives
            xmm0 = load_x(0, *blocks[0][1], chunks=[0, 10, 33])
            load_weights(range(1, R))
            transform(0, xmm0, 0, 4)
            transform(0, xmm0, 4, NPAIR)
            for g, (b, blk) in blocks[:-1]:
                for r in range(R):
                    if r == 1 and g + 1 < len(blocks):
                        nb, nblk = blocks[g + 1][1]
                        transform(g + 1, load_x(g + 1, nb, nblk))
                    if r == 2 and g > 0:
                        flush_block(g - 1, *blocks[g - 1][1])
                    for sp in range(NG):
                        conv_group(g, r, sp)
            # last block runs pair-group-major: each 8-row strip finishes all
            # rotations ~21us before the end, so its flush + output DMA hide
            # under the remaining strips' matmuls instead of trailing them
            g, (b, blk) = blocks[-1]
            for sp in range(NG):
                for r in range(R):
                    conv_group(g, r, sp)
                    if sp == 0 and r == 2:
                        flush_block(g - 1, *blocks[g - 1][1])
                # alternate queues so the two sliver transfers overlap
                flush_block(g, b, blk, 4 * sp, 4 * sp + 2,
                            eng=nc.sync if sp % 2 == 0 else nc.gpsimd)
                flush_block(g, b, blk, 4 * sp + 2, 4 * sp + 4,
                            eng=nc.gpsimd if sp % 2 == 0 else nc.sync)
    nc.finalize()
    return nc


def _get_nc():
    if "wino" not in _NC_CACHE:
        _NC_CACHE["wino"] = _build()
    return _NC_CACHE["wino"]


def _prep_weights(weight, rot_alpha):
    """Rotate the filter bank by the 8 angles and fold the vertical Winograd
    F(2,3) G-transform in; returns [R, CIN, 12*O] bf16."""
    M = _rot_mats(rot_alpha)
    w_r = (
        weight.reshape(O, R, CIN, 9).transpose(1, 0, 2, 3).astype(np.float64)
    )  # (R, O, I, 9)
    rot = np.einsum("rpq,roiq->roip", M.astype(np.float64), w_r)
    rot = rot.reshape(R, O, CIN, 3, 3)  # (ky, kx)
    G = np.array(
        [[1, 0, 0], [0.5, 0.5, 0.5], [0.5, -0.5, 0.5], [0, 0, 1]], np.float64
    )
    gp = np.einsum("jk,roikx->rijxo", G, rot)  # (R, I, 4, 3, O)
    return np.ascontiguousarray(
        gp.reshape(R, CIN, 12 * O).astype(np.float32).astype(BF16NP)
    )


def kernel(x, weight, rot_alpha):
    global LAST_RESULTS
    x = np.asarray(x, np.float32)
    weight = np.asarray(weight, np.float32)
    rot_alpha = np.asarray(rot_alpha, np.float32)

    wt = _prep_weights(weight, rot_alpha)
    xb = np.ascontiguousarray(x.astype(BF16NP))

    nc = _get_nc()
    in_maps = [
        {"xs": np.ascontiguousarray(xb[c * BL : (c + 1) * BL]), "wt": wt}
        for c in range(NCORES)
    ]
    try:
        res = run_bass_kernel_spmd(nc, in_maps, list(range(NCORES)), trace=_TRACE)
    except Exception:
        # One retry (without tracing): a failed compile or an aborted run can
        # leave a NeuronCore transiently wedged; the next attempt recovers.
        res = run_bass_kernel_spmd(nc, in_maps, list(range(NCORES)), trace=False)
    LAST_RESULTS = res
    return np.concatenate(
        [res.results[c]["y"] for c in range(NCORES)], axis=0
    ).astype(np.float32)
